# revision 4
# baseline (speedup 1.0000x reference)
"""Trainium2 Bass kernel for nn_Attention_81750407512209.

Full attention: out = softmax((x Wq)(x Wk)^T / sqrt(128)) @ (x Wv)
B=8 batches sharded 1:1 onto 8 NeuronCores (data parallel, weights replicated).

Per-core design (N=4096 ctx, D=128):
  - x^T via PE transpose; Q^T/K^T projections computed in float32r
    (~1.5e-4 matmul rel err measured on silicon) then stored bf16;
    1/sqrt(128) folded into Wq.  Scores matmul runs bf16 (2-byte moving
    operand streams at 1 cyc/row vs ~2.4 for 4-byte) - measured end-to-end
    rel err ~2e-3 vs the f32 reference.
  - Scores per 128-row q-tile in PSUM chunks (1536,1536,1024) - pool of
    two 3-bank slots + the 1024 chunk reuses a freed slot.
  - Row max via DVE reduce_max(negate=True) per chunk.
  - "Flash-lite" softmax: chunks 0,1 exponentiated with bias -max(c0,c1),
    chunk 2 with the full row -max; single PSUM rescale of the AV
    accumulator by gamma = exp(max01 - max) between AV kv-halves.
  - P = exp(S + bias) on ScalarE, PSUM -> SBUF bf16.
  - P^T via wide xbar DMA transposes ([128,2048] -> [128,16,128] batched
    block transpose) on the sync HWDGE engine only (xbar is a serialized
    resource; dual-engine issue corrupts data - measured).
  - AV: 32 bf16 matmuls lhsT=P^T tile [kv,q], rhs=V tile augmented with a
    ones column -> row sums accumulate in PSUM col 128.  Normalize with
    DVE reciprocal + ScalarE copy*scale.
  - Software pipelined: q-tile i-1's AV/normalize emitted interleaved with
    q-tile i's score work so PE is never blocked on the softmax chain.
"""

import numpy as np
from contextlib import ExitStack

import concourse.bass as bass
import concourse.tile as tile
from concourse import bacc, mybir
from concourse.bass_utils import run_bass_kernel_spmd
from concourse.masks import make_identity

F32 = mybir.dt.float32
F32R = mybir.dt.float32r
BF16 = mybir.dt.bfloat16
AX = mybir.AxisListType.X
OP = mybir.AluOpType
AF = mybir.ActivationFunctionType

B, N, D = 8, 4096, 128
NT = N // 128                    # 32 kv/q tiles
CHUNKS = (1536, 1536, 1024)      # score chunks; first two share bias m01
SCALE = 1.0 / np.sqrt(np.float32(D))
RESCALE_T = (CHUNKS[0] + CHUNKS[1]) // 128   # kv-tile where gamma applies (24)


def build_attention(nc: bacc.Bacc):
    x = nc.dram_tensor("x", [N, D], F32, kind="ExternalInput").ap()
    wq = nc.dram_tensor("w_query", [D, D], F32, kind="ExternalInput").ap()
    wk = nc.dram_tensor("w_key", [D, D], F32, kind="ExternalInput").ap()
    wv = nc.dram_tensor("w_value", [D, D], F32, kind="ExternalInput").ap()
    out = nc.dram_tensor("out", [N, D], F32, kind="ExternalOutput").ap()

    with tile.TileContext(nc) as tc, ExitStack() as ctx:
        consts = ctx.enter_context(tc.tile_pool(name="consts", bufs=1))
        big = ctx.enter_context(tc.tile_pool(name="big", bufs=1))
        xin = ctx.enter_context(tc.tile_pool(name="xin", bufs=4))
        pbuf = ctx.enter_context(tc.tile_pool(name="pbuf", bufs=2))
        stats = ctx.enter_context(tc.tile_pool(name="stats", bufs=3))
        ostage = ctx.enter_context(tc.tile_pool(name="ostage", bufs=3))

        ident = consts.tile([128, 128], F32, name="ident")
        make_identity(nc, ident[:])

        wq_st = consts.tile([128, 128], F32, name="wq_st")
        wk_st = consts.tile([128, 128], F32, name="wk_st")
        wv_st = consts.tile([128, 128], F32, name="wv_st")
        nc.sync.dma_start(wq_st[:], wq[:])
        nc.sync.dma_start(wk_st[:], wk[:])
        nc.sync.dma_start(wv_st[:], wv[:])
        wq_r = consts.tile([128, 128], F32R, name="wq_r")
        wk_r = consts.tile([128, 128], F32R, name="wk_r")
        wv_r = consts.tile([128, 128], F32R, name="wv_r")
        nc.vector.tensor_scalar_mul(wq_r[:], wq_st[:], float(SCALE))
        nc.vector.tensor_copy(wk_r[:], wk_st[:])
        nc.vector.tensor_copy(wv_r[:], wv_st[:])

        xT = big.tile([128, N], F32R, name="xT")
        kT = big.tile([128, N], BF16, name="kT")
        qT = big.tile([128, N], BF16, name="qT")
        vaug = big.tile([128, NT, 129], BF16, name="vaug")
        nc.gpsimd.memset(vaug[:, :, 128:129], 1.0)

        # ---- prologue: x^T, projections (scoped PSUM pool) ----
        with tc.tile_pool(name="ps_pro", bufs=2, space="PSUM") as ps_pro:
            for i in range(NT):
                xt = xin.tile([128, 128], F32, tag="xt", name="xt")
                nc.sync.dma_start(xt[:], x[i * 128:(i + 1) * 128, :])
                ps = ps_pro.tile([128, 128], F32, tag="xtp", name="xtp")
                nc.tensor.transpose(ps[:], xt[:], ident[:])
                nc.vector.tensor_copy(xT[:, i * 128:(i + 1) * 128], ps[:])
            for c in range(N // 512):
                sl = slice(c * 512, (c + 1) * 512)
                pk = ps_pro.tile([128, 512], F32, tag="proj", name="pk")
                nc.tensor.matmul(pk[:], wk_r[:], xT[:, sl], start=True, stop=True)
                nc.vector.tensor_copy(kT[:, sl], pk[:])
            for c in range(N // 512):
                sl = slice(c * 512, (c + 1) * 512)
                pq = ps_pro.tile([128, 512], F32, tag="proj", name="pq")
                nc.tensor.matmul(pq[:], wq_r[:], xT[:, sl], start=True, stop=True)
                nc.vector.tensor_copy(qT[:, sl], pq[:])
            for i in range(NT):
                pv = ps_pro.tile([128, 128], F32, tag="vproj", name="pv")
                nc.tensor.matmul(
                    pv[:], xT[:, i * 128:(i + 1) * 128], wv_r[:],
                    start=True, stop=True,
                )
                nc.scalar.copy(vaug[:, i, 0:128], pv[:])

        # ---- main loop pools: 2x3-bank score slots + 2x1-bank AV accum ----
        ps_s = ctx.enter_context(tc.tile_pool(name="ps_s", bufs=2, space="PSUM"))
        ps_av = ctx.enter_context(tc.tile_pool(name="ps_av", bufs=2, space="PSUM"))

        def score_chunk(qsl, off, width):
            s = ps_s.tile([128, CHUNKS[0]], F32, tag="sh", name="sh")
            for k in range(width // 512):
                nc.tensor.matmul(
                    s[:, k * 512:(k + 1) * 512],
                    qsl,
                    kT[:, off + k * 512: off + (k + 1) * 512],
                    start=True,
                    stop=True,
                )
            return s

        def negmax(s, width, tg):
            nm = stats.tile([128, 1], F32, tag=tg, name="nm")
            nc.vector.reduce_max(nm[:], s[:, 0:width], axis=AX, negate=True)
            return nm

        pend = None
        for i in range(NT + 1):
            # --- q-tile i: first two score chunks + their stats ---
            if i < NT:
                qsl = qT[:, i * 128:(i + 1) * 128]
                P = pbuf.tile([128, N], BF16, tag="P", name="P")
                PT = pbuf.tile([128, NT, 128], BF16, tag="PT", name="PT")

                s0 = score_chunk(qsl, 0, CHUNKS[0])
                s1 = score_chunk(qsl, CHUNKS[0], CHUNKS[1])
                n0 = negmax(s0, CHUNKS[0], "n0")
                n1 = negmax(s1, CHUNKS[1], "n1")
                b01 = stats.tile([128, 1], F32, tag="b01", name="b01")
                nc.vector.tensor_tensor(b01[:], n0[:], n1[:], op=OP.min)

            # --- q-tile i-1: AV + rescale first, so the rescale is at the
            # head of ScalarE's stream for this iteration ---
            if pend is not None:
                PTp, gamp, j = pend
                av = ps_av.tile([128, 129], F32, tag="av", name="av")
                for t in range(RESCALE_T):
                    nc.tensor.matmul(
                        av[:], PTp[:, t, :], vaug[:, t, :],
                        start=(t == 0), stop=False,
                    )
                # contributions so far were scaled with exp(-max01); bring to -max
                nc.scalar.activation(av[:], av[:], AF.Copy, bias=0.0, scale=gamp[:])
                for t in range(RESCALE_T, NT):
                    nc.tensor.matmul(
                        av[:], PTp[:, t, :], vaug[:, t, :],
                        start=False, stop=(t == NT - 1),
                    )
                linv = stats.tile([128, 1], F32, tag="linv", name="linv")
                nc.vector.reciprocal(linv[:], av[:, 128:129])

            # --- q-tile i: exps, last chunk, transposes ---
            if i < NT:
                nc.scalar.activation(P[:, 0:CHUNKS[0]], s0[:], AF.Exp, bias=b01[:])
                nc.scalar.activation(
                    P[:, CHUNKS[0]:CHUNKS[0] + CHUNKS[1]],
                    s1[:, 0:CHUNKS[1]], AF.Exp, bias=b01[:],
                )
                # first xbar transpose as soon as the first 2048 cols exist
                nc.sync.dma_start_transpose(PT[:, 0:16, :], P[:, 0:2048])

                s2 = score_chunk(qsl, CHUNKS[0] + CHUNKS[1], CHUNKS[2])
                n2 = negmax(s2, CHUNKS[2], "n2")
                bias = stats.tile([128, 1], F32, tag="bias", name="bias")
                nc.vector.tensor_tensor(bias[:], b01[:], n2[:], op=OP.min)
                gin = stats.tile([128, 1], F32, tag="gin", name="gin")
                nc.vector.tensor_tensor(gin[:], bias[:], b01[:], op=OP.subtract)
                gam = stats.tile([128, 1], F32, tag="gam", name="gam")
                nc.scalar.activation(gam[:], gin[:], AF.Exp)
                nc.scalar.activation(
                    P[:, CHUNKS[0] + CHUNKS[1]:N],
                    s2[:, 0:CHUNKS[2]], AF.Exp, bias=bias[:],
                )
                nc.sync.dma_start_transpose(PT[:, 16:32, :], P[:, 2048:4096])
                cur = (PT, gam, i)
            else:
                cur = None

            # --- q-tile i-1: normalize + store ---
            if pend is not None:
                ost = ostage.tile([128, 128], F32, tag="ost", name="ost")
                nc.scalar.activation(
                    ost[:], av[:, 0:128], AF.Copy, bias=0.0, scale=linv[:]
                )
                nc.sync.dma_start(out[j * 128:(j + 1) * 128, :], ost[:])
            pend = cur

    nc.compile()
    return nc


_NC_CACHE = {}


def _get_nc():
    if "nc" not in _NC_CACHE:
        nc = bacc.Bacc("TRN2", target_bir_lowering=False, debug=False, num_devices=B)
        _NC_CACHE["nc"] = build_attention(nc)
    return _NC_CACHE["nc"]


def kernel(x, w_query, w_key, w_value, _trace=False):
    x = np.ascontiguousarray(np.asarray(x, dtype=np.float32))
    w_query = np.ascontiguousarray(np.asarray(w_query, dtype=np.float32))
    w_key = np.ascontiguousarray(np.asarray(w_key, dtype=np.float32))
    w_value = np.ascontiguousarray(np.asarray(w_value, dtype=np.float32))
    nc = _get_nc()
    in_maps = [
        {"x": x[b], "w_query": w_query, "w_key": w_key, "w_value": w_value}
        for b in range(B)
    ]
    res = run_bass_kernel_spmd(nc, in_maps, core_ids=list(range(B)), trace=_trace)
    out_full = np.stack([res.results[b]["out"] for b in range(B)])
    if _trace:
        kernel.last_exec_time_ns = res.exec_time_ns
    return out_full


# revision 6
# speedup vs baseline: 1.0763x; 1.0763x over previous
"""Trainium2 Bass kernel for nn_Attention_81750407512209.

Full attention: out = softmax((x Wq)(x Wk)^T / sqrt(128)) @ (x Wv)
B=8 batches sharded 1:1 onto 8 NeuronCores (data parallel, weights replicated).

Per-core design (N=4096 ctx, D=128):
  - x^T via PE transpose; Q^T/K^T projections computed in float32r
    (~1.5e-4 matmul rel err measured on silicon) then stored bf16;
    1/sqrt(128) folded into Wq.  Scores matmul runs bf16 (2-byte moving
    operand streams at 1 cyc/row vs ~2.4 for 4-byte) - measured end-to-end
    rel err ~2e-3 vs the f32 reference.
  - Scores per 128-row q-tile in PSUM chunks (1536,1536,1024) - pool of
    two 3-bank slots + the 1024 chunk reuses a freed slot.
  - Row max via DVE reduce_max(negate=True) per chunk.
  - "Flash-lite" softmax: chunks 0,1 exponentiated with bias -max(c0,c1),
    chunk 2 with the full row -max; single PSUM rescale of the AV
    accumulator by gamma = exp(max01 - max) between AV kv-halves.
  - P = exp(S + bias) on ScalarE, PSUM -> SBUF bf16.
  - P^T via wide xbar DMA transposes ([128,2048] -> [128,16,128] batched
    block transpose) on the sync HWDGE engine only (xbar is a serialized
    resource; dual-engine issue corrupts data - measured).
  - AV: 32 bf16 matmuls lhsT=P^T tile [kv,q], rhs=V tile augmented with a
    ones column -> row sums accumulate in PSUM col 128.  Normalize with
    DVE reciprocal + ScalarE copy*scale.
  - Software pipelined: q-tile i-1's AV/normalize emitted interleaved with
    q-tile i's score work so PE is never blocked on the softmax chain.
"""

import numpy as np
from contextlib import ExitStack

import concourse.bass as bass
import concourse.tile as tile
from concourse import bacc, mybir
from concourse.bass_utils import run_bass_kernel_spmd
from concourse.masks import make_identity

F32 = mybir.dt.float32
F32R = mybir.dt.float32r
BF16 = mybir.dt.bfloat16
AX = mybir.AxisListType.X
OP = mybir.AluOpType
AF = mybir.ActivationFunctionType

B, N, D = 8, 4096, 128
NT = N // 128                    # 32 kv/q tiles
CHUNKS = (1536, 1536, 1024)      # score chunks; first two share bias m01
SCALE = 1.0 / np.sqrt(np.float32(D))
RESCALE_T = (CHUNKS[0] + CHUNKS[1]) // 128   # kv-tile where gamma applies (24)


def build_attention(nc: bacc.Bacc):
    x = nc.dram_tensor("x", [N, D], F32, kind="ExternalInput").ap()
    wq = nc.dram_tensor("w_query", [D, D], F32, kind="ExternalInput").ap()
    wk = nc.dram_tensor("w_key", [D, D], F32, kind="ExternalInput").ap()
    wv = nc.dram_tensor("w_value", [D, D], F32, kind="ExternalInput").ap()
    out = nc.dram_tensor("out", [N, D], F32, kind="ExternalOutput").ap()

    with tile.TileContext(nc) as tc, ExitStack() as ctx:
        consts = ctx.enter_context(tc.tile_pool(name="consts", bufs=1))
        big = ctx.enter_context(tc.tile_pool(name="big", bufs=1))
        xin = ctx.enter_context(tc.tile_pool(name="xin", bufs=4))
        pbuf = ctx.enter_context(tc.tile_pool(name="pbuf", bufs=3))
        stats = ctx.enter_context(tc.tile_pool(name="stats", bufs=3))
        ostage = ctx.enter_context(tc.tile_pool(name="ostage", bufs=3))

        ident = consts.tile([128, 128], F32, name="ident")
        make_identity(nc, ident[:])

        wq_st = consts.tile([128, 128], F32, name="wq_st")
        wk_st = consts.tile([128, 128], F32, name="wk_st")
        wv_st = consts.tile([128, 128], F32, name="wv_st")
        nc.sync.dma_start(wq_st[:], wq[:])
        nc.sync.dma_start(wk_st[:], wk[:])
        nc.sync.dma_start(wv_st[:], wv[:])
        wq_r = consts.tile([128, 128], F32R, name="wq_r")
        wk_r = consts.tile([128, 128], F32R, name="wk_r")
        wv_r = consts.tile([128, 128], F32R, name="wv_r")
        nc.vector.tensor_scalar_mul(wq_r[:], wq_st[:], float(SCALE))
        nc.vector.tensor_copy(wk_r[:], wk_st[:])
        nc.vector.tensor_copy(wv_r[:], wv_st[:])

        xT = big.tile([128, N], F32R, name="xT")
        kT = big.tile([128, N], BF16, name="kT")
        qT = big.tile([128, N], BF16, name="qT")
        vaug = big.tile([128, NT, 129], BF16, name="vaug")
        nc.gpsimd.memset(vaug[:, :, 128:129], 1.0)

        # ---- prologue: x^T, projections (scoped PSUM pool) ----
        with tc.tile_pool(name="ps_pro", bufs=2, space="PSUM") as ps_pro:
            for c in range(N // 512):
                sl = slice(c * 512, (c + 1) * 512)
                for u in range(4):
                    i = c * 4 + u
                    xt = xin.tile([128, 128], F32, tag="xt", name="xt")
                    nc.sync.dma_start(xt[:], x[i * 128:(i + 1) * 128, :])
                    ps = ps_pro.tile([128, 128], F32, tag="xtp", name="xtp")
                    nc.tensor.transpose(ps[:], xt[:], ident[:])
                    nc.vector.tensor_copy(xT[:, i * 128:(i + 1) * 128], ps[:])
                pk = ps_pro.tile([128, 512], F32, tag="proj", name="pk")
                nc.tensor.matmul(pk[:], wk_r[:], xT[:, sl], start=True, stop=True)
                nc.vector.tensor_copy(kT[:, sl], pk[:])
                pq = ps_pro.tile([128, 512], F32, tag="proj", name="pq")
                nc.tensor.matmul(pq[:], wq_r[:], xT[:, sl], start=True, stop=True)
                nc.vector.tensor_copy(qT[:, sl], pq[:])
                for u in range(4):
                    i = c * 4 + u
                    pv = ps_pro.tile([128, 128], F32, tag="vproj", name="pv")
                    nc.tensor.matmul(
                        pv[:], xT[:, i * 128:(i + 1) * 128], wv_r[:],
                        start=True, stop=True,
                    )
                    nc.scalar.copy(vaug[:, i, 0:128], pv[:])

        # ---- main loop pools: 2x3-bank score slots + 2x1-bank AV accum ----
        ps_s = ctx.enter_context(tc.tile_pool(name="ps_s", bufs=2, space="PSUM"))
        ps_av = ctx.enter_context(tc.tile_pool(name="ps_av", bufs=2, space="PSUM"))

        def score_chunk(qsl, off, width):
            s = ps_s.tile([128, CHUNKS[0]], F32, tag="sh", name="sh")
            for k in range(width // 512):
                nc.tensor.matmul(
                    s[:, k * 512:(k + 1) * 512],
                    qsl,
                    kT[:, off + k * 512: off + (k + 1) * 512],
                    start=True,
                    stop=True,
                )
            return s

        def negmax(s, width, tg):
            nm = stats.tile([128, 1], F32, tag=tg, name="nm")
            nc.vector.reduce_max(nm[:], s[:, 0:width], axis=AX, negate=True)
            return nm

        t1 = None  # tile awaiting AV first half
        t2 = None  # tile awaiting rescale + AV second half + normalize
        for i in range(NT + 2):
            # A: tile i - score chunks 0,1 and their maxes
            if i < NT:
                qsl = qT[:, i * 128:(i + 1) * 128]
                P = pbuf.tile([128, N], BF16, tag="P", name="P")
                PT = pbuf.tile([128, NT, 128], BF16, tag="PT", name="PT")
                s0 = score_chunk(qsl, 0, CHUNKS[0])
                s1 = score_chunk(qsl, CHUNKS[0], CHUNKS[1])
                n0 = negmax(s0, CHUNKS[0], "n0")
                n1 = negmax(s1, CHUNKS[1], "n1")
                b01 = stats.tile([128, 1], F32, tag="b01", name="b01")
                nc.vector.tensor_tensor(b01[:], n0[:], n1[:], op=OP.min)

            # B: tile i-1 - AV over the first RESCALE_T kv-tiles
            if t1 is not None:
                PT1, gam1, j1 = t1
                av1 = ps_av.tile([128, 129], F32, tag="av", name="av")
                for t in range(RESCALE_T):
                    nc.tensor.matmul(
                        av1[:], PT1[:, t, :], vaug[:, t, :],
                        start=(t == 0), stop=False,
                    )

            # C: tile i-2 - rescale (ScalarE head-of-stream; its AV first
            # half ran last iteration), AV tail, reciprocal
            if t2 is not None:
                PT2, gam2, av2, j2 = t2
                nc.scalar.activation(av2[:], av2[:], AF.Copy, bias=0.0, scale=gam2[:])
                for t in range(RESCALE_T, NT):
                    nc.tensor.matmul(
                        av2[:], PT2[:, t, :], vaug[:, t, :],
                        start=False, stop=(t == NT - 1),
                    )
                linv = stats.tile([128, 1], F32, tag="linv", name="linv")
                nc.vector.reciprocal(linv[:], av2[:, 128:129])

            # D: tile i - exps, last score chunk, xbar transposes
            if i < NT:
                nc.scalar.activation(P[:, 0:CHUNKS[0]], s0[:], AF.Exp, bias=b01[:])
                nc.scalar.activation(
                    P[:, CHUNKS[0]:CHUNKS[0] + CHUNKS[1]],
                    s1[:, 0:CHUNKS[1]], AF.Exp, bias=b01[:],
                )
                # kv-tiles 0..23 (= chunks 0,1) in one batched block-transpose
                nc.sync.dma_start_transpose(
                    PT[:, 0:RESCALE_T, :], P[:, 0:RESCALE_T * 128]
                )
                s2 = score_chunk(qsl, CHUNKS[0] + CHUNKS[1], CHUNKS[2])
                n2 = negmax(s2, CHUNKS[2], "n2")
                bias = stats.tile([128, 1], F32, tag="bias", name="bias")
                nc.vector.tensor_tensor(bias[:], b01[:], n2[:], op=OP.min)
                gin = stats.tile([128, 1], F32, tag="gin", name="gin")
                nc.vector.tensor_tensor(gin[:], bias[:], b01[:], op=OP.subtract)
                gam = stats.tile([128, 1], F32, tag="gam", name="gam")
                nc.scalar.activation(gam[:], gin[:], AF.Exp)
                nc.scalar.activation(
                    P[:, CHUNKS[0] + CHUNKS[1]:N],
                    s2[:, 0:CHUNKS[2]], AF.Exp, bias=bias[:],
                )
                nc.sync.dma_start_transpose(
                    PT[:, RESCALE_T:NT, :], P[:, RESCALE_T * 128:N]
                )

            # E: tile i-2 - normalize and store
            if t2 is not None:
                ost = ostage.tile([128, 128], F32, tag="ost", name="ost")
                nc.scalar.activation(
                    ost[:], av2[:, 0:128], AF.Copy, bias=0.0, scale=linv[:]
                )
                nc.sync.dma_start(out[j2 * 128:(j2 + 1) * 128, :], ost[:])

            t2 = (t1[0], t1[1], av1, t1[2]) if t1 is not None else None
            t1 = (PT, gam, i) if i < NT else None

    nc.compile()
    return nc


_NC_CACHE = {}


def _get_nc():
    if "nc" not in _NC_CACHE:
        nc = bacc.Bacc("TRN2", target_bir_lowering=False, debug=False, num_devices=B)
        _NC_CACHE["nc"] = build_attention(nc)
    return _NC_CACHE["nc"]


def kernel(x, w_query, w_key, w_value, _trace=False):
    x = np.ascontiguousarray(np.asarray(x, dtype=np.float32))
    w_query = np.ascontiguousarray(np.asarray(w_query, dtype=np.float32))
    w_key = np.ascontiguousarray(np.asarray(w_key, dtype=np.float32))
    w_value = np.ascontiguousarray(np.asarray(w_value, dtype=np.float32))
    nc = _get_nc()
    in_maps = [
        {"x": x[b], "w_query": w_query, "w_key": w_key, "w_value": w_value}
        for b in range(B)
    ]
    res = run_bass_kernel_spmd(nc, in_maps, core_ids=list(range(B)), trace=_trace)
    out_full = np.stack([res.results[b]["out"] for b in range(B)])
    if _trace:
        kernel.last_exec_time_ns = res.exec_time_ns
    return out_full
